# revision 12
# baseline (speedup 1.0000x reference)
"""MoE encoder-decoder transformer on 8 TRN2 NeuronCores (Bass/Tile SPMD).

Sharding: token-parallel dense compute (256 tokens/core), expert-parallel MoE
(1 expert/core, AllGather + ReduceScatter), vocab-sharded final projection
(4000 cols/core), pair-AllGather of activations for attention K/V (computed
locally for the full 512-token batch). Activations kept feature-major
[D, tok] in SBUF so all matmuls are transpose-free; partition-axis
reductions (LN mean/var, softmax sums) via ones-vector matmuls and
outer-product broadcasts on the PE.
"""

import numpy as np

import concourse.bacc as bacc
import concourse.tile as tile
from concourse import mybir, bass_utils

# ---- model dims (hardcoded per problem spec) ----
B, S, T, D, H, DH, F, E, L, V = 4, 512, 512, 512, 8, 64, 2048, 8, 2, 32000
NCORES = 8
TOK = 256            # tokens per core
VS = V // NCORES     # vocab shard
KT = D // 128        # 4 feature k-tiles
FT = F // 128        # 16 ffn k-tiles
NEGM = -30000.0      # additive mask value (exp underflows to 0 in f32)
EPS = 1e-6

MM_DT = mybir.dt.bfloat16   # matmul dtype: bfloat16 | float32 | float32r
F32 = mybir.dt.float32

PAIR_GROUPS = [[0, 1], [2, 3], [4, 5], [6, 7]]
ALL_GROUPS = [list(range(NCORES))]
Act = mybir.ActivationFunctionType


def _pos_encoding(length, d):
    half = d // 2
    pos = np.arange(length, dtype=np.float32)[:, None]
    rates = 1.0 / (10000.0 ** (np.arange(half, dtype=np.float32) / half))
    ang = pos * rates
    return np.concatenate([np.sin(ang), np.cos(ang)], axis=-1)


# ======================================================================
# device program
# ======================================================================

def build_nc():
    nc = bacc.Bacc("TRN2", target_bir_lowering=False, debug=False,
                   num_devices=NCORES)

    def inp(name, shape, dt=F32):
        return nc.dram_tensor(name, shape, dt, kind="ExternalInput")

    x0 = inp("x0", [D, TOK])            # encoder emb+PE, feature-major
    y0 = inp("y0", [D, TOK])            # decoder emb+PE, feature-major
    ekb = inp("ekb", [S, 1])            # encoder key additive mask
    dmask = inp("dmask", [T, TOK])      # decoder self-attn maskT (k, q_local)

    # wmha = [wq*0.125 | wk | wv | wo] along cols; bmha likewise [4D,1].
    wmha = {s: inp(f"wmha_{s}", [L, D, 4 * D], MM_DT) for s in ("e", "d1", "d2")}
    bmha = {s: inp(f"bmha_{s}", [L, 4 * D, 1]) for s in ("e", "d1", "d2")}
    wf1 = {s: inp(f"wf1_{s}", [L, D, F], MM_DT) for s in ("e", "d")}
    bf1 = {s: inp(f"bf1_{s}", [L, F, 1]) for s in ("e", "d")}
    wf2 = {s: inp(f"wf2_{s}", [L, F, D], MM_DT) for s in ("e", "d")}
    bf2 = {s: inp(f"bf2_{s}", [L, D, 1]) for s in ("e", "d")}
    lng = inp("lng", [12, D, 1])        # [enc(g1,g2,moe) x2, dec(g1,g2,g3) x2]
    lnb = inp("lnb", [12, D, 1])
    gw = inp("gw", [L, D, E], MM_DT)    # gate cols host-rolled: col0 = my expert
    gb = inp("gb", [L, E, 1])
    mw1 = inp("mw1", [L, D, F], MM_DT)  # my expert
    mb1 = inp("mb1", [L, F, 1])
    mw2 = inp("mw2", [L, F, D], MM_DT)
    mb2 = inp("mb2", [L, D, 1])
    fw = inp("fw", [D, VS], MM_DT)      # vocab shard
    fb = inp("fb", [1, VS])

    out = nc.dram_tensor("out", [NCORES * TOK, VS], F32, kind="ExternalOutput")

    with tile.TileContext(nc, pool_alloc_mode="queue") as tc:
        from contextlib import ExitStack
        top = ExitStack()
        const = top.enter_context(tc.tile_pool(name="const", bufs=1))
        actp = top.enter_context(tc.tile_pool(name="actp", bufs=2))
        castp = top.enter_context(tc.tile_pool(name="castp", bufs=2))
        pairp = top.enter_context(tc.tile_pool(name="pairp", bufs=2))
        pp = top.enter_context(tc.tile_pool(name="pp", bufs=3, space="PSUM"))
        ppr = top.enter_context(tc.tile_pool(name="ppr", bufs=2, space="PSUM"))
        pph = top.enter_context(tc.tile_pool(name="pph", bufs=2, space="PSUM"))

        # ---- constants ----
        ones128_f = const.tile([128, 1], F32, tag="o128f", name="o128f")
        nc.vector.memset(ones128_f[:], 1.0)
        ones128_m = const.tile([128, 1], MM_DT, tag="o128m", name="o128m")
        nc.vector.memset(ones128_m[:], 1.0)
        ones1_f = const.tile([1, 128], F32, tag="o1f", name="o1f")
        nc.vector.memset(ones1_f[:], 1.0)
        ones8_f = const.tile([8, 1], F32, tag="o8f", name="o8f")
        nc.vector.memset(ones8_f[:], 1.0)
        eps_t = const.tile([1, 1], F32, tag="epsc", name="epsc")
        nc.vector.memset(eps_t[:], EPS)
        ones1_m = const.tile([1, 128], MM_DT, tag="o1m", name="o1m")
        nc.vector.memset(ones1_m[:], 1.0)
        ones8_m = const.tile([8, 1], MM_DT, tag="o8m", name="o8m")
        nc.vector.memset(ones8_m[:], 1.0)
        heat_src = const.tile([128, 512], MM_DT, tag="heatsrc", name="heatsrc")
        nc.vector.memset(heat_src[:], 0.25)
        pheat = top.enter_context(tc.tile_pool(name="pheat", bufs=1,
                                               space="PSUM"))

        def heat(n):
            """Self-paced dummy matmul chain (PE->DVE->PE ...) that keeps the
            PE HAM clock warm across multi-us stalls without racing ahead."""
            for _ in range(n):
                hp_ = pheat.tile([1, 512], F32, tag="ht", name="ht")
                nc.tensor.matmul(hp_[:, :], lhsT=ones128_m[:, :],
                                 rhs=heat_src[:, :], start=True, stop=True)
                nc.vector.tensor_copy(heat_src[0:1, :], hp_[:, :])

        def psb(n=512):
            return pp.tile([128, n], F32, tag="pb", name="pb")

        def psrow(parts=1, n=512):
            return ppr.tile([parts, n], F32, tag="pr", name="pr")

        def load_cols(p, dram2d, n, tagpfx, dt=F32):
            """[n,1] DRAM slice -> one [128, n/128] tile; return column views."""
            nk = n // 128
            t = p.tile([128, nk], dt, tag=f"{tagpfx}", name=f"{tagpfx}")
            nc.sync.dma_start(
                t[:, :], dram2d.rearrange("(k p) o -> p (k o)", p=128))
            return [t[:, k:k + 1] for k in range(nk)]

        def fm_matmul(p, w_sb, col0, x_tiles, mtiles, bias_tiles=None,
                      act=None, out_dt=None, tagpfx="fm"):
            """out^T[m] = act(W[:, col0+m*128]^T @ x (+ b)); x_tiles k-major."""
            n = x_tiles[0].shape[1]
            nk = len(x_tiles)
            outs = []
            for m in range(mtiles):
                ps = psb(n)
                for k in range(nk):
                    nc.tensor.matmul(
                        ps[:, :],
                        lhsT=w_sb[k][:, col0 + m * 128:col0 + (m + 1) * 128],
                        rhs=x_tiles[k][:, :],
                        start=(k == 0), stop=(k == nk - 1))
                o = p.tile([128, n], out_dt or MM_DT, tag=f"{tagpfx}{m}", name=f"{tagpfx}{m}")
                fn = act or Act.Identity
                b = bias_tiles[m][:, :] if bias_tiles is not None else 0.0
                nc.scalar.activation(o[:, :], ps[:, :], fn, bias=b)
                outs.append(o)
            return outs

        def cast_mm(x_tiles, fam):
            if MM_DT == F32:
                return x_tiles
            outs = []
            for k, t in enumerate(x_tiles):
                o = castp.tile([128, t.shape[1]], MM_DT, tag=f"cm_{fam}{k}", name=f"cm_{fam}{k}")
                nc.scalar.copy(o[:, :], t[:, :])
                outs.append(o)
            return outs

        def layernorm(p, z_tiles, ln_idx, ofam):
            """LN over partition axis (D, KT tiles [128,n] f32) -> actp tiles."""
            n = z_tiles[0].shape[1]
            g_t = load_cols(p, lng[ln_idx], D, "lg")
            b_t = load_cols(p, lnb[ln_idx], D, "lb")
            psm = psrow(1, n)
            psq = psrow(1, n)
            zmm, sq = [], []
            for k in range(KT):
                zm = p.tile([128, n], MM_DT, tag=f"zm{k}", name=f"zm{k}")
                nc.vector.tensor_copy(zm[:, :], z_tiles[k][:, :])
                zmm.append(zm)
                s = p.tile([128, n], MM_DT, tag=f"sq{k}", name=f"sq{k}")
                nc.vector.tensor_mul(s[:, :], z_tiles[k][:, :], z_tiles[k][:, :])
                sq.append(s)
            for k in range(KT):
                nc.tensor.matmul(psm[:, :], lhsT=ones128_m[:, :],
                                 rhs=zmm[k][:, :], start=(k == 0),
                                 stop=(k == KT - 1))
            for k in range(KT):
                nc.tensor.matmul(psq[:, :], lhsT=ones128_m[:, :],
                                 rhs=sq[k][:, :], start=(k == 0),
                                 stop=(k == KT - 1))
            mean = p.tile([1, n], F32, tag="mn", name="mn")
            nc.scalar.mul(mean[:, :], psm[:, :], 1.0 / D)
            var = p.tile([1, n], F32, tag="vr", name="vr")
            nc.scalar.mul(var[:, :], psq[:, :], 1.0 / D)
            m2 = p.tile([1, n], F32, tag="m2", name="m2")
            nc.vector.tensor_mul(m2[:, :], mean[:, :], mean[:, :])
            nc.vector.tensor_sub(var[:, :], var[:, :], m2[:, :])
            sd = p.tile([1, n], F32, tag="sd", name="sd")
            nc.scalar.activation(sd[:, :], var[:, :], Act.Sqrt, bias=eps_t[:, :])
            rstd = p.tile([1, n], F32, tag="rs", name="rs")
            nc.vector.reciprocal(rstd[:, :], sd[:, :])
            rstd_m = p.tile([1, n], MM_DT, tag="rsm", name="rsm")
            nc.vector.tensor_copy(rstd_m[:, :], rstd[:, :])
            m2_m = p.tile([1, n], MM_DT, tag="m2m", name="m2m")
            nc.vector.tensor_mul(m2_m[:, :], mean[:, :], rstd[:, :])
            psA = psb(n)
            nc.tensor.matmul(psA[:, :], lhsT=ones1_m[:, :], rhs=rstd_m[:, :],
                             start=True, stop=True)
            A = p.tile([128, n], F32, tag="lnA", name="lnA")
            nc.vector.tensor_copy(A[:, :], psA[:, :])
            psM = psb(n)
            nc.tensor.matmul(psM[:, :], lhsT=ones1_m[:, :], rhs=m2_m[:, :],
                             start=True, stop=True)
            Mb = p.tile([128, n], F32, tag="lnM", name="lnM")
            nc.vector.tensor_copy(Mb[:, :], psM[:, :])
            outs = []
            for k in range(KT):
                t = p.tile([128, n], F32, tag=f"lt{k}", name=f"lt{k}")
                nc.vector.tensor_mul(t[:, :], z_tiles[k][:, :], A[:, :])
                nc.vector.tensor_sub(t[:, :], t[:, :], Mb[:, :])
                o = actp.tile([128, n], F32, tag=f"{ofam}{k}", name=f"{ofam}{k}")
                nc.vector.tensor_scalar(o[:, :], t[:, :], g_t[k][:, :],
                                        b_t[k][:, :], mybir.AluOpType.mult,
                                        mybir.AluOpType.add)
                outs.append(o)
            return outs

        def pair_allgather(x_mm, fam):
            """Pair-AG KT [128,TOK] MM tiles -> KT [128,2*TOK] pairp tiles."""
            cin = nc.dram_tensor(f"agp_{fam}_in", [D, TOK], MM_DT)
            cout = nc.dram_tensor(f"agp_{fam}_out", [2 * D, TOK], MM_DT)
            for k in range(KT):
                nc.sync.dma_start(cin[k * 128:(k + 1) * 128, :], x_mm[k][:, :])
            nc.gpsimd.collective_compute(
                "AllGather", mybir.AluOpType.bypass, replica_groups=PAIR_GROUPS,
                ins=[cin.ap().opt()], outs=[cout.ap().opt()])
            heat(10)
            pair = []
            for k in range(KT):
                t = pairp.tile([128, 2 * TOK], MM_DT, tag=f"pair_{fam}{k}", name=f"pair_{fam}{k}")
                nc.sync.dma_start(t[:, 0:TOK], cout[k * 128:(k + 1) * 128, :])
                nc.sync.dma_start(t[:, TOK:2 * TOK],
                                  cout[D + k * 128:D + (k + 1) * 128, :])
                pair.append(t)
            return pair

        def mha(x_f32, x_mm, pair_mm, wset, layer, kbias_tiles, dmask_tiles,
                ln_idx, ofam):
            with tc.tile_pool(name="mhap", bufs=1) as p, \
                 tc.tile_pool(name="mhap2", bufs=2) as p2:
                w_sb = []
                for k in range(KT):
                    t = p.tile([128, 4 * D], MM_DT, tag=f"wm{k}", name=f"wm{k}")
                    nc.sync.dma_start(
                        t[:, :], wmha[wset][layer, k * 128:(k + 1) * 128, :])
                    w_sb.append(t)
                bq = load_cols(p, bmha[wset][layer, 0:D, :], D, "bq")
                bk = load_cols(p, bmha[wset][layer, D:2 * D, :], D, "bk")
                bo = load_cols(p, bmha[wset][layer, 3 * D:4 * D, :], D, "bo")
                bvf = p.tile([1, D], F32, tag="bvf", name="bvf")
                nc.sync.dma_start(
                    bvf[:, :],
                    bmha[wset][layer, 2 * D:3 * D, :].rearrange("a b -> b a"))
                bv = p.tile([1, D], MM_DT, tag="bv", name="bv")
                nc.vector.tensor_copy(bv[:, :], bvf[:, :])

                SKV = pair_mm[0].shape[1]      # gathered batch tokens (512)
                KKT = SKV // 128
                qT = fm_matmul(p, w_sb, 0, x_mm, KT, bias_tiles=bq,
                               tagpfx="qT")
                kT = fm_matmul(p, w_sb, D, pair_mm, KT, bias_tiles=bk,
                               tagpfx="kT")
                v = []
                for th in range(KKT):
                    psv = psb(D)
                    for k in range(KT):
                        nc.tensor.matmul(
                            psv[:, :],
                            lhsT=pair_mm[k][:, th * 128:(th + 1) * 128],
                            rhs=w_sb[k][:, 2 * D:3 * D],
                            start=(k == 0), stop=False)
                    nc.tensor.matmul(psv[:, :], lhsT=ones1_m[:, :],
                                     rhs=bv[:, :], start=False, stop=True)
                    vt = p.tile([128, D], MM_DT, tag=f"v{th}", name=f"v{th}")
                    nc.vector.tensor_copy(vt[:, :], psv[:, :])
                    v.append(vt)

                oT = [p.tile([128, TOK], MM_DT, tag=f"oT{m}", name=f"oT{m}")
                      for m in range(KT)]
                for h in range(H):
                    hp, ho = h // 2, (h % 2) * 64
                    att = []
                    for kt in range(KKT):
                        ps = psb(TOK)
                        nc.tensor.matmul(
                            ps[:, :],
                            lhsT=kT[hp][ho:ho + 64, kt * 128:(kt + 1) * 128],
                            rhs=qT[hp][ho:ho + 64, :],
                            start=True, stop=True)
                        if dmask_tiles is not None:
                            nc.vector.tensor_add(ps[:, :], ps[:, :],
                                                 dmask_tiles[kt][:, :])
                        a = p2.tile([128, TOK], MM_DT, tag=f"att{kt}", name=f"att{kt}")
                        bias = (kbias_tiles[kt][:, :]
                                if kbias_tiles is not None else 0.0)
                        nc.scalar.activation(a[:, :], ps[:, :], Act.Exp,
                                             bias=bias)
                        att.append(a)
                    psr = psrow(1, TOK)
                    for kt in range(KKT):
                        nc.tensor.matmul(psr[:, :], lhsT=ones128_m[:, :],
                                         rhs=att[kt][:, :], start=(kt == 0),
                                         stop=(kt == KKT - 1))
                    rsum = p2.tile([1, TOK], F32, tag="rsum", name="rsum")
                    nc.scalar.copy(rsum[:, :], psr[:, :])
                    rec = p2.tile([1, TOK], F32, tag="rec", name="rec")
                    nc.vector.reciprocal(rec[:, :], rsum[:, :])
                    rec_m = p2.tile([1, TOK], MM_DT, tag="recm", name="recm")
                    nc.vector.tensor_copy(rec_m[:, :], rec[:, :])
                    pso = pph.tile([64, TOK], F32, tag="ph", name="ph")
                    for kt in range(KKT):
                        nc.tensor.matmul(pso[:, :],
                                         lhsT=v[kt][:, h * 64:(h + 1) * 64],
                                         rhs=att[kt][:, :], start=(kt == 0),
                                         stop=(kt == KKT - 1))
                    psc = pph.tile([64, TOK], F32, tag="ph", name="ph")
                    nc.tensor.matmul(psc[:, :], lhsT=ones1_m[0:1, 0:64],
                                     rhs=rec_m[:, :], start=True, stop=True)
                    bc = p2.tile([64, TOK], F32, tag="bc", name="bc")
                    nc.vector.tensor_copy(bc[:, :], psc[:, :])
                    nc.vector.tensor_mul(oT[hp][ho:ho + 64, :], pso[:, :],
                                         bc[:, :])
                z = []
                for m in range(KT):
                    ps = psb(TOK)
                    for k in range(KT):
                        nc.tensor.matmul(
                            ps[:, :],
                            lhsT=w_sb[k][:, 3 * D + m * 128:3 * D + (m + 1) * 128],
                            rhs=oT[k][:, :], start=(k == 0), stop=(k == KT - 1))
                    t = p.tile([128, TOK], F32, tag=f"z{m}", name=f"z{m}")
                    nc.scalar.activation(t[:, :], ps[:, :], Act.Identity,
                                         bias=bo[m][:, :])
                    nc.vector.tensor_add(t[:, :], t[:, :], x_f32[m][:, :])
                    z.append(t)
                return layernorm(p, z, ln_idx, ofam)

        def ffn(x_f32, x_mm, fset, layer, ln_idx, ofam):
            with tc.tile_pool(name="ffnp", bufs=1) as p, \
                 tc.tile_pool(name="ffnp2", bufs=2) as p2:
                w1_sb = []
                for k in range(KT):
                    t = p.tile([128, F], MM_DT, tag=f"w1_{k}", name=f"w1_{k}")
                    nc.sync.dma_start(
                        t[:, :], wf1[fset][layer, k * 128:(k + 1) * 128, :])
                    w1_sb.append(t)
                b1 = load_cols(p, bf1[fset][layer], F, "b1")
                h_sb = fm_matmul(p, w1_sb, 0, x_mm, FT, bias_tiles=b1,
                                 act=Act.Relu, tagpfx="hf")
                w2_sb = []
                for k in range(FT):
                    t = p.tile([128, D], MM_DT, tag=f"w2_{k}", name=f"w2_{k}")
                    nc.sync.dma_start(
                        t[:, :], wf2[fset][layer, k * 128:(k + 1) * 128, :])
                    w2_sb.append(t)
                b2 = load_cols(p, bf2[fset][layer], D, "b2")
                z = []
                for m in range(KT):
                    ps = psb(TOK)
                    for k in range(FT):
                        nc.tensor.matmul(
                            ps[:, :], lhsT=w2_sb[k][:, m * 128:(m + 1) * 128],
                            rhs=h_sb[k][:, :], start=(k == 0),
                            stop=(k == FT - 1))
                    t = p.tile([128, TOK], F32, tag=f"z{m}", name=f"z{m}")
                    nc.scalar.activation(t[:, :], ps[:, :], Act.Identity,
                                         bias=b2[m][:, :])
                    nc.vector.tensor_add(t[:, :], t[:, :], x_f32[m][:, :])
                    z.append(t)
                return layernorm(p, z, ln_idx, ofam)

        def moe(x_f32, x_mm, layer, ln_idx, ofam):
            agin = nc.dram_tensor(f"agm{layer}_in", [D, TOK], MM_DT)
            agout = nc.dram_tensor(f"agm{layer}_out", [NCORES * D, TOK], MM_DT,
                                   addr_space="Shared")
            rsin = nc.dram_tensor(f"rsm{layer}_in", [NCORES * D, TOK], F32)
            rsout = nc.dram_tensor(f"rsm{layer}_out", [D, TOK], F32)
            for k in range(KT):
                nc.sync.dma_start(agin[k * 128:(k + 1) * 128, :], x_mm[k][:, :])
            nc.gpsimd.collective_compute(
                "AllGather", mybir.AluOpType.bypass, replica_groups=ALL_GROUPS,
                ins=[agin.ap().opt()], outs=[agout.ap().opt()])
            heat(14)
            with tc.tile_pool(name="moep", bufs=1) as p, \
                 tc.tile_pool(name="moep2", bufs=2) as p2:
                gw_sb, w1_sb, w2_sb = [], [], []
                for k in range(KT):
                    t = p.tile([128, E], MM_DT, tag=f"gw{k}", name=f"gw{k}")
                    nc.sync.dma_start(t[:, :],
                                      gw[layer, k * 128:(k + 1) * 128, :])
                    gw_sb.append(t)
                    t = p.tile([128, F], MM_DT, tag=f"e1_{k}", name=f"e1_{k}")
                    nc.sync.dma_start(t[:, :],
                                      mw1[layer, k * 128:(k + 1) * 128, :])
                    w1_sb.append(t)
                for k in range(FT):
                    t = p.tile([128, D], MM_DT, tag=f"e2_{k}", name=f"e2_{k}")
                    nc.sync.dma_start(t[:, :],
                                      mw2[layer, k * 128:(k + 1) * 128, :])
                    w2_sb.append(t)
                gb_sb = p.tile([8, 1], F32, tag="gb", name="gb")
                nc.sync.dma_start(gb_sb[:, :], gb[layer])
                b1 = load_cols(p, mb1[layer], F, "m1")
                b2 = load_cols(p, mb2[layer], D, "m2")

                NCH = 2 * TOK
                for j in range(NCORES * TOK // NCH):
                    r0, r1 = 2 * j, 2 * j + 1
                    xall = []
                    for k in range(KT):
                        t = p2.tile([128, NCH], MM_DT, tag=f"xa{k}", name=f"xa{k}")
                        nc.sync.dma_start(
                            t[:, 0:TOK],
                            agout[r0 * D + k * 128:r0 * D + (k + 1) * 128, :])
                        nc.sync.dma_start(
                            t[:, TOK:NCH],
                            agout[r1 * D + k * 128:r1 * D + (k + 1) * 128, :])
                        xall.append(t)
                    psg = psrow(8, NCH)
                    for k in range(KT):
                        nc.tensor.matmul(psg[:, :], lhsT=gw_sb[k][:, :],
                                         rhs=xall[k][:, :], start=(k == 0),
                                         stop=(k == KT - 1))
                    eg = p2.tile([8, NCH], MM_DT, tag="eg", name="eg")
                    nc.scalar.activation(eg[:, :], psg[:, :], Act.Exp,
                                         bias=gb_sb[:, :])
                    psgs = psrow(1, NCH)
                    nc.tensor.matmul(psgs[:, :], lhsT=ones8_m[:, :],
                                     rhs=eg[:, :], start=True, stop=True)
                    gs = p2.tile([1, NCH], F32, tag="gs", name="gs")
                    nc.scalar.copy(gs[:, :], psgs[:, :])
                    grec = p2.tile([1, NCH], F32, tag="grec", name="grec")
                    nc.vector.reciprocal(grec[:, :], gs[:, :])
                    gmy = p2.tile([1, NCH], MM_DT, tag="gmy", name="gmy")
                    nc.vector.tensor_mul(gmy[:, :], eg[0:1, :], grec[:, :])
                    psG = psb(NCH)
                    nc.tensor.matmul(psG[:, :], lhsT=ones1_m[:, :],
                                     rhs=gmy[:, :], start=True, stop=True)
                    G = p2.tile([128, NCH], F32, tag="G", name="G")
                    nc.vector.tensor_copy(G[:, :], psG[:, :])
                    h_sb = fm_matmul(p2, w1_sb, 0, xall, FT, bias_tiles=b1,
                                     act=Act.Relu, tagpfx="mh")
                    for m in range(KT):
                        ps = psb(NCH)
                        for k in range(FT):
                            nc.tensor.matmul(
                                ps[:, :],
                                lhsT=w2_sb[k][:, m * 128:(m + 1) * 128],
                                rhs=h_sb[k][:, :], start=(k == 0),
                                stop=(k == FT - 1))
                        t = p2.tile([128, NCH], F32, tag=f"ye{m}", name=f"ye{m}")
                        nc.scalar.activation(t[:, :], ps[:, :], Act.Identity,
                                             bias=b2[m][:, :])
                        nc.vector.tensor_mul(t[:, :], t[:, :], G[:, :])
                        nc.sync.dma_start(
                            rsin[r0 * D + m * 128:r0 * D + (m + 1) * 128, :],
                            t[:, 0:TOK])
                        nc.sync.dma_start(
                            rsin[r1 * D + m * 128:r1 * D + (m + 1) * 128, :],
                            t[:, TOK:NCH])
                nc.gpsimd.collective_compute(
                    "ReduceScatter", mybir.AluOpType.add,
                    replica_groups=ALL_GROUPS,
                    ins=[rsin.ap().opt()], outs=[rsout.ap().opt()])
                heat(30)
                z = []
                for k in range(KT):
                    t = p.tile([128, TOK], F32, tag=f"z{k}", name=f"z{k}")
                    nc.sync.dma_start(t[:, :], rsout[k * 128:(k + 1) * 128, :])
                    nc.vector.tensor_add(t[:, :], t[:, :], x_f32[k][:, :])
                    z.append(t)
                return layernorm(p, z, ln_idx, ofam)

        # ================= program =================
        heat(16)
        x = []
        for k in range(KT):
            t = actp.tile([128, TOK], F32, tag=f"x{k}", name=f"x{k}")
            nc.sync.dma_start(t[:, :], x0[k * 128:(k + 1) * 128, :])
            x.append(t)
        y = []
        for k in range(KT):
            t = actp.tile([128, TOK], F32, tag=f"y{k}", name=f"y{k}")
            nc.sync.dma_start(t[:, :], y0[k * 128:(k + 1) * 128, :])
            y.append(t)
        ekb_sb = []
        for k in range(S // 128):
            t = const.tile([128, 1], F32, tag=f"ekb{k}", name=f"ekb{k}")
            nc.sync.dma_start(t[:, :], ekb[k * 128:(k + 1) * 128, :])
            ekb_sb.append(t)
        dmask_sb = []
        for k in range(T // 128):
            t = const.tile([128, TOK], F32, tag=f"dm{k}", name=f"dm{k}")
            nc.sync.dma_start(t[:, :], dmask[k * 128:(k + 1) * 128, :])
            dmask_sb.append(t)

        # encoder
        for i in range(L):
            x_mm = cast_mm(x, "a")
            pair = pair_allgather(x_mm, f"e{i}")
            x = mha(x, x_mm, pair, "e", i, ekb_sb, None, 3 * i + 0, "x")
            x_mm = cast_mm(x, "b")
            x = ffn(x, x_mm, "e", i, 3 * i + 1, "x")
            x_mm = cast_mm(x, "a")
            x = moe(x, x_mm, i, 3 * i + 2, "x")

        # decoder
        enc_mm = cast_mm(x, "b")
        enc_pair = pair_allgather(enc_mm, "en")
        for i in range(L):
            y_mm = cast_mm(y, "a")
            ypair = pair_allgather(y_mm, f"d{i}")
            y = mha(y, y_mm, ypair, "d1", i, None, dmask_sb, 6 + 3 * i, "y")
            y_mm = cast_mm(y, "b")
            y = mha(y, y_mm, enc_pair, "d2", i, ekb_sb, None, 6 + 3 * i + 1,
                    "y")
            y_mm = cast_mm(y, "a")
            y = ffn(y, y_mm, "d", i, 6 + 3 * i + 2, "y")

        # final projection (vocab shard)
        y_mm = cast_mm(y, "b")
        fagin = nc.dram_tensor("fag_in", [D, TOK], MM_DT)
        fagout = nc.dram_tensor("fag_out", [NCORES * D, TOK], MM_DT,
                                addr_space="Shared")
        for k in range(KT):
            nc.sync.dma_start(fagin[k * 128:(k + 1) * 128, :], y_mm[k][:, :])
        nc.gpsimd.collective_compute(
            "AllGather", mybir.AluOpType.bypass, replica_groups=ALL_GROUPS,
            ins=[fagin.ap().opt()], outs=[fagout.ap().opt()])
        heat(14)
        with tc.tile_pool(name="finp", bufs=1) as p, \
             tc.tile_pool(name="finp2", bufs=2) as p2, \
             tc.tile_pool(name="finp3", bufs=4) as p3:
            fw_sb = []
            for k in range(KT):
                t = p.tile([128, VS], MM_DT, tag=f"fw{k}", name=f"fw{k}")
                nc.sync.dma_start(t[:, :], fw[k * 128:(k + 1) * 128, :])
                fw_sb.append(t)
            fbf = p.tile([1, VS], F32, tag="fbf", name="fbf")
            nc.sync.dma_start(fbf[:, :], fb[:, :])
            fb_sb = p.tile([1, VS], MM_DT, tag="fb", name="fb")
            nc.vector.tensor_copy(fb_sb[:, :], fbf[:, :])
            NV = 500
            for j in range(4):
                r0, r1 = 2 * j, 2 * j + 1
                yall = []
                for k in range(KT):
                    t = p2.tile([128, 2 * TOK], MM_DT, tag=f"ya{k}", name=f"ya{k}")
                    nc.sync.dma_start(
                        t[:, 0:TOK],
                        fagout[r0 * D + k * 128:r0 * D + (k + 1) * 128, :])
                    nc.sync.dma_start(
                        t[:, TOK:2 * TOK],
                        fagout[r1 * D + k * 128:r1 * D + (k + 1) * 128, :])
                    yall.append(t)
                for q in range(4):
                    orow = p3.tile([128, VS], F32, tag="of", name="of")
                    for n in range(VS // NV):
                        ps = psb(NV)
                        for k in range(KT):
                            nc.tensor.matmul(
                                ps[:, :],
                                lhsT=yall[k][:, q * 128:(q + 1) * 128],
                                rhs=fw_sb[k][:, n * NV:(n + 1) * NV],
                                start=(k == 0), stop=False)
                        nc.tensor.matmul(ps[:, :], lhsT=ones1_m[:, :],
                                         rhs=fb_sb[:, n * NV:(n + 1) * NV],
                                         start=False, stop=True)
                        nc.vector.tensor_copy(
                            orow[:, n * NV:(n + 1) * NV], ps[:, :])
                    nc.sync.dma_start(
                        out[j * 512 + q * 128:j * 512 + (q + 1) * 128, :],
                        orow[:, :])
        top.close()

    nc.compile()
    return nc


# ======================================================================
# host side
# ======================================================================

def _np(x):
    return np.asarray(x, dtype=np.float32)


def make_in_maps(inp, tar, params):
    inp = np.asarray(inp)
    tar = np.asarray(tar)
    pe = _pos_encoding(1000, D)
    xe = _np(params["emb_enc"])[inp] + pe[:S][None]      # [B,S,D]
    yd = _np(params["emb_dec"])[tar] + pe[:T][None]
    xe = xe.reshape(B * S, D)
    yd = yd.reshape(B * T, D)

    mmnp = (np.float32 if MM_DT in (F32, mybir.dt.float32r)
            else np.dtype("bfloat16"))

    def mm(a):
        return np.ascontiguousarray(_np(a)).astype(mmnp)

    def col(a):
        return np.ascontiguousarray(_np(a)[..., None])

    def pack_mha(p):
        w = np.concatenate([_np(p["wq"]) * 0.125, _np(p["wk"]), _np(p["wv"]),
                            _np(p["wo"])], axis=2)        # [L,D,4D]
        b = np.concatenate([_np(p["bq"]) * 0.125, _np(p["bk"]), _np(p["bv"]),
                            _np(p["bo"])], axis=1)        # [L,4D]
        return mm(w), col(b)

    wm_e, bm_e = pack_mha(params["enc_mha"])
    wm_d1, bm_d1 = pack_mha(params["dec_mha1"])
    wm_d2, bm_d2 = pack_mha(params["dec_mha2"])

    eln, dln, m = params["enc_ln"], params["dec_ln"], params["moe"]
    lng_ = np.stack([eln["g1"][0], eln["g2"][0], m["lng"][0],
                     eln["g1"][1], eln["g2"][1], m["lng"][1],
                     dln["g1"][0], dln["g2"][0], dln["g3"][0],
                     dln["g1"][1], dln["g2"][1], dln["g3"][1]])
    lnb_ = np.stack([eln["b1"][0], eln["b2"][0], m["lnb"][0],
                     eln["b1"][1], eln["b2"][1], m["lnb"][1],
                     dln["b1"][0], dln["b2"][0], dln["b3"][0],
                     dln["b1"][1], dln["b2"][1], dln["b3"][1]])

    look = 1.0 - np.tril(np.ones((T, T), np.float32))
    common = {
        "wmha_e": wm_e, "bmha_e": bm_e,
        "wmha_d1": wm_d1, "bmha_d1": bm_d1,
        "wmha_d2": wm_d2, "bmha_d2": bm_d2,
        "wf1_e": mm(params["enc_ffn"]["w1"]), "bf1_e": col(params["enc_ffn"]["b1"]),
        "wf2_e": mm(params["enc_ffn"]["w2"]), "bf2_e": col(params["enc_ffn"]["b2"]),
        "wf1_d": mm(params["dec_ffn"]["w1"]), "bf1_d": col(params["dec_ffn"]["b1"]),
        "wf2_d": mm(params["dec_ffn"]["w2"]), "bf2_d": col(params["dec_ffn"]["b2"]),
        "lng": col(lng_), "lnb": col(lnb_),
    }
    in_maps = []
    for c in range(NCORES):
        b, half = c // 2, c % 2
        rows = slice(c * TOK, (c + 1) * TOK)
        enc_pad = (inp[b] == 0).astype(np.float32)        # [S]
        dec_pad = (tar[b] == 0).astype(np.float32)        # [T]
        q0 = half * TOK
        comb = np.maximum(dec_pad[None, :], look[q0:q0 + TOK])  # [TOK,T]
        perm = (np.arange(E) + c) % E
        im = dict(common)
        im.update({
            "x0": np.ascontiguousarray(xe[rows].T),
            "y0": np.ascontiguousarray(yd[rows].T),
            "ekb": np.ascontiguousarray((enc_pad * NEGM)[:, None]),
            "dmask": np.ascontiguousarray((comb * NEGM).T),
            "gw": mm(_np(m["gw"])[:, :, perm]),
            "gb": col(_np(m["gb"])[:, perm]),
            "mw1": mm(_np(m["w1"])[:, c]),
            "mb1": col(_np(m["b1"])[:, c]),
            "mw2": mm(_np(m["w2"])[:, c]),
            "mb2": col(_np(m["b2"])[:, c]),
            "fw": mm(_np(params["final_w"])[:, c * VS:(c + 1) * VS]),
            "fb": np.ascontiguousarray(
                _np(params["final_b"])[None, c * VS:(c + 1) * VS]),
        })
        in_maps.append(im)
    return in_maps


_CACHED_NC = None


def get_nc():
    global _CACHED_NC
    if _CACHED_NC is None:
        _CACHED_NC = build_nc()
    return _CACHED_NC


def run(inp, tar, params, trace=False, tmpdir=None):
    nc = get_nc()
    in_maps = make_in_maps(inp, tar, params)
    res = bass_utils.run_bass_kernel_spmd(
        nc, in_maps, core_ids=list(range(NCORES)), trace=trace, tmpdir=tmpdir)
    full = np.concatenate([res.results[c]["out"] for c in range(NCORES)],
                          axis=1)                         # [2048, V]
    return np.ascontiguousarray(full.reshape(B, T, V)), res


def kernel(inp, tar, params):
    return run(inp, tar, params)[0]


# revision 13
# speedup vs baseline: 1.0784x; 1.0784x over previous
"""MoE encoder-decoder transformer on 8 TRN2 NeuronCores (Bass/Tile SPMD).

Sharding: token-parallel dense compute (256 tokens/core), expert-parallel MoE
(1 expert/core, AllGather + ReduceScatter), vocab-sharded final projection
(4000 cols/core), pair-AllGather of activations for attention K/V (computed
locally for the full 512-token batch). Activations kept feature-major
[D, tok] in SBUF so all matmuls are transpose-free; partition-axis
reductions (LN mean/var, softmax sums) via ones-vector matmuls and
outer-product broadcasts on the PE.
"""

import numpy as np

import concourse.bacc as bacc
import concourse.tile as tile
from concourse import mybir, bass_utils

# ---- model dims (hardcoded per problem spec) ----
B, S, T, D, H, DH, F, E, L, V = 4, 512, 512, 512, 8, 64, 2048, 8, 2, 32000
NCORES = 8
TOK = 256            # tokens per core
VS = V // NCORES     # vocab shard
KT = D // 128        # 4 feature k-tiles
FT = F // 128        # 16 ffn k-tiles
NEGM = -30000.0      # additive mask value (exp underflows to 0 in f32)
EPS = 1e-6

MM_DT = mybir.dt.bfloat16   # matmul dtype: bfloat16 | float32 | float32r
F32 = mybir.dt.float32

PAIR_GROUPS = [[0, 1], [2, 3], [4, 5], [6, 7]]
ALL_GROUPS = [list(range(NCORES))]
Act = mybir.ActivationFunctionType


def _pos_encoding(length, d):
    half = d // 2
    pos = np.arange(length, dtype=np.float32)[:, None]
    rates = 1.0 / (10000.0 ** (np.arange(half, dtype=np.float32) / half))
    ang = pos * rates
    return np.concatenate([np.sin(ang), np.cos(ang)], axis=-1)


# ======================================================================
# device program
# ======================================================================

def build_nc():
    nc = bacc.Bacc("TRN2", target_bir_lowering=False, debug=False,
                   num_devices=NCORES)

    def inp(name, shape, dt=F32):
        return nc.dram_tensor(name, shape, dt, kind="ExternalInput")

    x0 = inp("x0", [D, TOK])            # encoder emb+PE, feature-major
    y0 = inp("y0", [D, TOK])            # decoder emb+PE, feature-major
    ekb = inp("ekb", [S, 1])            # encoder key additive mask
    dmask = inp("dmask", [T, TOK])      # decoder self-attn maskT (k, q_local)

    # wmha = [wq*0.125 | wk | wv | wo] along cols; bmha likewise [4D,1].
    wmha = {s: inp(f"wmha_{s}", [L, D, 4 * D], MM_DT) for s in ("e", "d1", "d2")}
    bmha = {s: inp(f"bmha_{s}", [L, 4 * D, 1]) for s in ("e", "d1", "d2")}
    wf1 = {s: inp(f"wf1_{s}", [L, D, F], MM_DT) for s in ("e", "d")}
    bf1 = {s: inp(f"bf1_{s}", [L, F, 1]) for s in ("e", "d")}
    wf2 = {s: inp(f"wf2_{s}", [L, F, D], MM_DT) for s in ("e", "d")}
    bf2 = {s: inp(f"bf2_{s}", [L, D, 1]) for s in ("e", "d")}
    lng = inp("lng", [12, D, 1])        # [enc(g1,g2,moe) x2, dec(g1,g2,g3) x2]
    lnb = inp("lnb", [12, D, 1])
    gw = inp("gw", [L, D, E], MM_DT)    # gate cols host-rolled: col0 = my expert
    gb = inp("gb", [L, E, 1])
    mw1 = inp("mw1", [L, D, F], MM_DT)  # my expert
    mb1 = inp("mb1", [L, F, 1])
    mw2 = inp("mw2", [L, F, D], MM_DT)
    mb2 = inp("mb2", [L, D, 1])
    fw = inp("fw", [D, VS], MM_DT)      # vocab shard
    fb = inp("fb", [1, VS])

    out = nc.dram_tensor("out", [NCORES * TOK, VS], F32, kind="ExternalOutput")

    with tile.TileContext(nc, pool_alloc_mode="queue") as tc:
        from contextlib import ExitStack
        top = ExitStack()
        const = top.enter_context(tc.tile_pool(name="const", bufs=1))
        actp = top.enter_context(tc.tile_pool(name="actp", bufs=2))
        castp = top.enter_context(tc.tile_pool(name="castp", bufs=2))
        pairp = top.enter_context(tc.tile_pool(name="pairp", bufs=2))
        pp = top.enter_context(tc.tile_pool(name="pp", bufs=3, space="PSUM"))
        ppr = top.enter_context(tc.tile_pool(name="ppr", bufs=2, space="PSUM"))
        pph = top.enter_context(tc.tile_pool(name="pph", bufs=2, space="PSUM"))

        # ---- constants ----
        ones128_f = const.tile([128, 1], F32, tag="o128f", name="o128f")
        nc.vector.memset(ones128_f[:], 1.0)
        ones128_m = const.tile([128, 1], MM_DT, tag="o128m", name="o128m")
        nc.vector.memset(ones128_m[:], 1.0)
        ones1_f = const.tile([1, 128], F32, tag="o1f", name="o1f")
        nc.vector.memset(ones1_f[:], 1.0)
        ones8_f = const.tile([8, 1], F32, tag="o8f", name="o8f")
        nc.vector.memset(ones8_f[:], 1.0)
        eps_t = const.tile([1, 1], F32, tag="epsc", name="epsc")
        nc.vector.memset(eps_t[:], EPS)
        ones1_m = const.tile([1, 128], MM_DT, tag="o1m", name="o1m")
        nc.vector.memset(ones1_m[:], 1.0)
        ones8_m = const.tile([8, 1], MM_DT, tag="o8m", name="o8m")
        nc.vector.memset(ones8_m[:], 1.0)
        heat_src = const.tile([128, 512], MM_DT, tag="heatsrc", name="heatsrc")
        nc.vector.memset(heat_src[:], 0.25)
        pheat = top.enter_context(tc.tile_pool(name="pheat", bufs=1,
                                               space="PSUM"))

        def heat(n):
            """Self-paced dummy matmul chain (PE->DVE->PE ...) that keeps the
            PE HAM clock warm across multi-us stalls without racing ahead."""
            for _ in range(n):
                hp_ = pheat.tile([1, 512], F32, tag="ht", name="ht")
                nc.tensor.matmul(hp_[:, :], lhsT=ones128_m[:, :],
                                 rhs=heat_src[:, :], start=True, stop=True)

        def psb(n=512):
            return pp.tile([128, n], F32, tag="pb", name="pb")

        def psrow(parts=1, n=512):
            return ppr.tile([parts, n], F32, tag="pr", name="pr")

        def load_cols(p, dram2d, n, tagpfx, dt=F32):
            """[n,1] DRAM slice -> one [128, n/128] tile; return column views."""
            nk = n // 128
            t = p.tile([128, nk], dt, tag=f"{tagpfx}", name=f"{tagpfx}")
            nc.sync.dma_start(
                t[:, :], dram2d.rearrange("(k p) o -> p (k o)", p=128))
            return [t[:, k:k + 1] for k in range(nk)]

        def fm_matmul(p, w_sb, col0, x_tiles, mtiles, bias_tiles=None,
                      act=None, out_dt=None, tagpfx="fm"):
            """out^T[m] = act(W[:, col0+m*128]^T @ x (+ b)); x_tiles k-major."""
            n = x_tiles[0].shape[1]
            nk = len(x_tiles)
            outs = []
            for m in range(mtiles):
                ps = psb(n)
                for k in range(nk):
                    nc.tensor.matmul(
                        ps[:, :],
                        lhsT=w_sb[k][:, col0 + m * 128:col0 + (m + 1) * 128],
                        rhs=x_tiles[k][:, :],
                        start=(k == 0), stop=(k == nk - 1))
                o = p.tile([128, n], out_dt or MM_DT, tag=f"{tagpfx}{m}", name=f"{tagpfx}{m}")
                fn = act or Act.Identity
                b = bias_tiles[m][:, :] if bias_tiles is not None else 0.0
                nc.scalar.activation(o[:, :], ps[:, :], fn, bias=b)
                outs.append(o)
            return outs

        def cast_mm(x_tiles, fam):
            if MM_DT == F32:
                return x_tiles
            outs = []
            for k, t in enumerate(x_tiles):
                o = castp.tile([128, t.shape[1]], MM_DT, tag=f"cm_{fam}{k}", name=f"cm_{fam}{k}")
                nc.scalar.copy(o[:, :], t[:, :])
                outs.append(o)
            return outs

        def layernorm(p, z_tiles, ln_idx, ofam):
            """LN over partition axis (D, KT tiles [128,n] f32) -> actp tiles."""
            n = z_tiles[0].shape[1]
            g_t = load_cols(p, lng[ln_idx], D, "lg")
            b_t = load_cols(p, lnb[ln_idx], D, "lb")
            psm = psrow(1, n)
            psq = psrow(1, n)
            zmm, sq = [], []
            for k in range(KT):
                zm = p.tile([128, n], MM_DT, tag=f"zm{k}", name=f"zm{k}")
                nc.vector.tensor_copy(zm[:, :], z_tiles[k][:, :])
                zmm.append(zm)
                s = p.tile([128, n], MM_DT, tag=f"sq{k}", name=f"sq{k}")
                nc.vector.tensor_mul(s[:, :], z_tiles[k][:, :], z_tiles[k][:, :])
                sq.append(s)
            for k in range(KT):
                nc.tensor.matmul(psm[:, :], lhsT=ones128_m[:, :],
                                 rhs=zmm[k][:, :], start=(k == 0),
                                 stop=(k == KT - 1))
            for k in range(KT):
                nc.tensor.matmul(psq[:, :], lhsT=ones128_m[:, :],
                                 rhs=sq[k][:, :], start=(k == 0),
                                 stop=(k == KT - 1))
            mean = p.tile([1, n], F32, tag="mn", name="mn")
            nc.scalar.mul(mean[:, :], psm[:, :], 1.0 / D)
            var = p.tile([1, n], F32, tag="vr", name="vr")
            nc.scalar.mul(var[:, :], psq[:, :], 1.0 / D)
            m2 = p.tile([1, n], F32, tag="m2", name="m2")
            nc.vector.tensor_mul(m2[:, :], mean[:, :], mean[:, :])
            nc.vector.tensor_sub(var[:, :], var[:, :], m2[:, :])
            sd = p.tile([1, n], F32, tag="sd", name="sd")
            nc.scalar.activation(sd[:, :], var[:, :], Act.Sqrt, bias=eps_t[:, :])
            rstd = p.tile([1, n], F32, tag="rs", name="rs")
            nc.vector.reciprocal(rstd[:, :], sd[:, :])
            rstd_m = p.tile([1, n], MM_DT, tag="rsm", name="rsm")
            nc.vector.tensor_copy(rstd_m[:, :], rstd[:, :])
            m2_m = p.tile([1, n], MM_DT, tag="m2m", name="m2m")
            nc.vector.tensor_mul(m2_m[:, :], mean[:, :], rstd[:, :])
            psA = psb(n)
            nc.tensor.matmul(psA[:, :], lhsT=ones1_m[:, :], rhs=rstd_m[:, :],
                             start=True, stop=True)
            A = p.tile([128, n], F32, tag="lnA", name="lnA")
            nc.vector.tensor_copy(A[:, :], psA[:, :])
            psM = psb(n)
            nc.tensor.matmul(psM[:, :], lhsT=ones1_m[:, :], rhs=m2_m[:, :],
                             start=True, stop=True)
            Mb = p.tile([128, n], F32, tag="lnM", name="lnM")
            nc.vector.tensor_copy(Mb[:, :], psM[:, :])
            outs = []
            for k in range(KT):
                t = p.tile([128, n], F32, tag=f"lt{k}", name=f"lt{k}")
                nc.vector.tensor_mul(t[:, :], z_tiles[k][:, :], A[:, :])
                nc.vector.tensor_sub(t[:, :], t[:, :], Mb[:, :])
                o = actp.tile([128, n], F32, tag=f"{ofam}{k}", name=f"{ofam}{k}")
                nc.vector.tensor_scalar(o[:, :], t[:, :], g_t[k][:, :],
                                        b_t[k][:, :], mybir.AluOpType.mult,
                                        mybir.AluOpType.add)
                outs.append(o)
            return outs

        def pair_allgather(x_mm, fam):
            """Pair-AG KT [128,TOK] MM tiles -> KT [128,2*TOK] pairp tiles."""
            cin = nc.dram_tensor(f"agp_{fam}_in", [D, TOK], MM_DT)
            cout = nc.dram_tensor(f"agp_{fam}_out", [2 * D, TOK], MM_DT)
            for k in range(KT):
                nc.sync.dma_start(cin[k * 128:(k + 1) * 128, :], x_mm[k][:, :])
            nc.gpsimd.collective_compute(
                "AllGather", mybir.AluOpType.bypass, replica_groups=PAIR_GROUPS,
                ins=[cin.ap().opt()], outs=[cout.ap().opt()])
            heat(10)
            pair = []
            for k in range(KT):
                t = pairp.tile([128, 2 * TOK], MM_DT, tag=f"pair_{fam}{k}", name=f"pair_{fam}{k}")
                nc.sync.dma_start(t[:, 0:TOK], cout[k * 128:(k + 1) * 128, :])
                nc.sync.dma_start(t[:, TOK:2 * TOK],
                                  cout[D + k * 128:D + (k + 1) * 128, :])
                pair.append(t)
            return pair

        def mha(x_f32, x_mm, pair_mm, wset, layer, kbias_tiles, dmask_tiles,
                ln_idx, ofam):
            with tc.tile_pool(name="mhap", bufs=1) as p, \
                 tc.tile_pool(name="mhap2", bufs=2) as p2:
                w_sb = []
                for k in range(KT):
                    t = p.tile([128, 4 * D], MM_DT, tag=f"wm{k}", name=f"wm{k}")
                    nc.sync.dma_start(
                        t[:, :], wmha[wset][layer, k * 128:(k + 1) * 128, :])
                    w_sb.append(t)
                bq = load_cols(p, bmha[wset][layer, 0:D, :], D, "bq")
                bk = load_cols(p, bmha[wset][layer, D:2 * D, :], D, "bk")
                bo = load_cols(p, bmha[wset][layer, 3 * D:4 * D, :], D, "bo")
                bvf = p.tile([1, D], F32, tag="bvf", name="bvf")
                nc.sync.dma_start(
                    bvf[:, :],
                    bmha[wset][layer, 2 * D:3 * D, :].rearrange("a b -> b a"))
                bv = p.tile([1, D], MM_DT, tag="bv", name="bv")
                nc.vector.tensor_copy(bv[:, :], bvf[:, :])

                SKV = pair_mm[0].shape[1]      # gathered batch tokens (512)
                KKT = SKV // 128
                qT = fm_matmul(p, w_sb, 0, x_mm, KT, bias_tiles=bq,
                               tagpfx="qT")
                kT = fm_matmul(p, w_sb, D, pair_mm, KT, bias_tiles=bk,
                               tagpfx="kT")
                v = []
                for th in range(KKT):
                    psv = psb(D)
                    for k in range(KT):
                        nc.tensor.matmul(
                            psv[:, :],
                            lhsT=pair_mm[k][:, th * 128:(th + 1) * 128],
                            rhs=w_sb[k][:, 2 * D:3 * D],
                            start=(k == 0), stop=False)
                    nc.tensor.matmul(psv[:, :], lhsT=ones1_m[:, :],
                                     rhs=bv[:, :], start=False, stop=True)
                    vt = p.tile([128, D], MM_DT, tag=f"v{th}", name=f"v{th}")
                    nc.vector.tensor_copy(vt[:, :], psv[:, :])
                    v.append(vt)

                oT = [p.tile([128, TOK], MM_DT, tag=f"oT{m}", name=f"oT{m}")
                      for m in range(KT)]
                for h in range(H):
                    hp, ho = h // 2, (h % 2) * 64
                    att = []
                    for kt in range(KKT):
                        ps = psb(TOK)
                        nc.tensor.matmul(
                            ps[:, :],
                            lhsT=kT[hp][ho:ho + 64, kt * 128:(kt + 1) * 128],
                            rhs=qT[hp][ho:ho + 64, :],
                            start=True, stop=True)
                        if dmask_tiles is not None:
                            nc.vector.tensor_add(ps[:, :], ps[:, :],
                                                 dmask_tiles[kt][:, :])
                        a = p2.tile([128, TOK], MM_DT, tag=f"att{kt}", name=f"att{kt}")
                        bias = (kbias_tiles[kt][:, :]
                                if kbias_tiles is not None else 0.0)
                        nc.scalar.activation(a[:, :], ps[:, :], Act.Exp,
                                             bias=bias)
                        att.append(a)
                    psr = psrow(1, TOK)
                    for kt in range(KKT):
                        nc.tensor.matmul(psr[:, :], lhsT=ones128_m[:, :],
                                         rhs=att[kt][:, :], start=(kt == 0),
                                         stop=(kt == KKT - 1))
                    rsum = p2.tile([1, TOK], F32, tag="rsum", name="rsum")
                    nc.scalar.copy(rsum[:, :], psr[:, :])
                    rec = p2.tile([1, TOK], F32, tag="rec", name="rec")
                    nc.vector.reciprocal(rec[:, :], rsum[:, :])
                    rec_m = p2.tile([1, TOK], MM_DT, tag="recm", name="recm")
                    nc.vector.tensor_copy(rec_m[:, :], rec[:, :])
                    pso = pph.tile([64, TOK], F32, tag="ph", name="ph")
                    for kt in range(KKT):
                        nc.tensor.matmul(pso[:, :],
                                         lhsT=v[kt][:, h * 64:(h + 1) * 64],
                                         rhs=att[kt][:, :], start=(kt == 0),
                                         stop=(kt == KKT - 1))
                    psc = pph.tile([64, TOK], F32, tag="ph", name="ph")
                    nc.tensor.matmul(psc[:, :], lhsT=ones1_m[0:1, 0:64],
                                     rhs=rec_m[:, :], start=True, stop=True)
                    bc = p2.tile([64, TOK], F32, tag="bc", name="bc")
                    nc.vector.tensor_copy(bc[:, :], psc[:, :])
                    nc.vector.tensor_mul(oT[hp][ho:ho + 64, :], pso[:, :],
                                         bc[:, :])
                z = []
                for m in range(KT):
                    ps = psb(TOK)
                    for k in range(KT):
                        nc.tensor.matmul(
                            ps[:, :],
                            lhsT=w_sb[k][:, 3 * D + m * 128:3 * D + (m + 1) * 128],
                            rhs=oT[k][:, :], start=(k == 0), stop=(k == KT - 1))
                    t = p.tile([128, TOK], F32, tag=f"z{m}", name=f"z{m}")
                    nc.scalar.activation(t[:, :], ps[:, :], Act.Identity,
                                         bias=bo[m][:, :])
                    nc.vector.tensor_add(t[:, :], t[:, :], x_f32[m][:, :])
                    z.append(t)
                return layernorm(p, z, ln_idx, ofam)

        def ffn(x_f32, x_mm, fset, layer, ln_idx, ofam):
            with tc.tile_pool(name="ffnp", bufs=1) as p, \
                 tc.tile_pool(name="ffnp2", bufs=2) as p2:
                w1_sb = []
                for k in range(KT):
                    t = p.tile([128, F], MM_DT, tag=f"w1_{k}", name=f"w1_{k}")
                    nc.sync.dma_start(
                        t[:, :], wf1[fset][layer, k * 128:(k + 1) * 128, :])
                    w1_sb.append(t)
                b1 = load_cols(p, bf1[fset][layer], F, "b1")
                h_sb = fm_matmul(p, w1_sb, 0, x_mm, FT, bias_tiles=b1,
                                 act=Act.Relu, tagpfx="hf")
                w2_sb = []
                for k in range(FT):
                    t = p.tile([128, D], MM_DT, tag=f"w2_{k}", name=f"w2_{k}")
                    nc.sync.dma_start(
                        t[:, :], wf2[fset][layer, k * 128:(k + 1) * 128, :])
                    w2_sb.append(t)
                b2 = load_cols(p, bf2[fset][layer], D, "b2")
                z = []
                for m in range(KT):
                    ps = psb(TOK)
                    for k in range(FT):
                        nc.tensor.matmul(
                            ps[:, :], lhsT=w2_sb[k][:, m * 128:(m + 1) * 128],
                            rhs=h_sb[k][:, :], start=(k == 0),
                            stop=(k == FT - 1))
                    t = p.tile([128, TOK], F32, tag=f"z{m}", name=f"z{m}")
                    nc.scalar.activation(t[:, :], ps[:, :], Act.Identity,
                                         bias=b2[m][:, :])
                    nc.vector.tensor_add(t[:, :], t[:, :], x_f32[m][:, :])
                    z.append(t)
                return layernorm(p, z, ln_idx, ofam)

        def moe(x_f32, x_mm, layer, ln_idx, ofam):
            agin = nc.dram_tensor(f"agm{layer}_in", [D, TOK], MM_DT)
            agout = nc.dram_tensor(f"agm{layer}_out", [NCORES * D, TOK], MM_DT,
                                   addr_space="Shared")
            rsin = nc.dram_tensor(f"rsm{layer}_in", [NCORES * D, TOK], MM_DT)
            rsout = nc.dram_tensor(f"rsm{layer}_out", [D, TOK], MM_DT)
            for k in range(KT):
                nc.sync.dma_start(agin[k * 128:(k + 1) * 128, :], x_mm[k][:, :])
            nc.gpsimd.collective_compute(
                "AllGather", mybir.AluOpType.bypass, replica_groups=ALL_GROUPS,
                ins=[agin.ap().opt()], outs=[agout.ap().opt()])
            heat(14)
            with tc.tile_pool(name="moep", bufs=1) as p, \
                 tc.tile_pool(name="moep2", bufs=2) as p2:
                gw_sb, w1_sb, w2_sb = [], [], []
                for k in range(KT):
                    t = p.tile([128, E], MM_DT, tag=f"gw{k}", name=f"gw{k}")
                    nc.sync.dma_start(t[:, :],
                                      gw[layer, k * 128:(k + 1) * 128, :])
                    gw_sb.append(t)
                    t = p.tile([128, F], MM_DT, tag=f"e1_{k}", name=f"e1_{k}")
                    nc.sync.dma_start(t[:, :],
                                      mw1[layer, k * 128:(k + 1) * 128, :])
                    w1_sb.append(t)
                for k in range(FT):
                    t = p.tile([128, D], MM_DT, tag=f"e2_{k}", name=f"e2_{k}")
                    nc.sync.dma_start(t[:, :],
                                      mw2[layer, k * 128:(k + 1) * 128, :])
                    w2_sb.append(t)
                gb_sb = p.tile([8, 1], F32, tag="gb", name="gb")
                nc.sync.dma_start(gb_sb[:, :], gb[layer])
                b1 = load_cols(p, mb1[layer], F, "m1")
                b2 = load_cols(p, mb2[layer], D, "m2")

                NCH = 2 * TOK
                for j in range(NCORES * TOK // NCH):
                    r0, r1 = 2 * j, 2 * j + 1
                    xall = []
                    for k in range(KT):
                        t = p2.tile([128, NCH], MM_DT, tag=f"xa{k}", name=f"xa{k}")
                        nc.sync.dma_start(
                            t[:, 0:TOK],
                            agout[r0 * D + k * 128:r0 * D + (k + 1) * 128, :])
                        nc.sync.dma_start(
                            t[:, TOK:NCH],
                            agout[r1 * D + k * 128:r1 * D + (k + 1) * 128, :])
                        xall.append(t)
                    psg = psrow(8, NCH)
                    for k in range(KT):
                        nc.tensor.matmul(psg[:, :], lhsT=gw_sb[k][:, :],
                                         rhs=xall[k][:, :], start=(k == 0),
                                         stop=(k == KT - 1))
                    eg = p2.tile([8, NCH], MM_DT, tag="eg", name="eg")
                    nc.scalar.activation(eg[:, :], psg[:, :], Act.Exp,
                                         bias=gb_sb[:, :])
                    psgs = psrow(1, NCH)
                    nc.tensor.matmul(psgs[:, :], lhsT=ones8_m[:, :],
                                     rhs=eg[:, :], start=True, stop=True)
                    gs = p2.tile([1, NCH], F32, tag="gs", name="gs")
                    nc.scalar.copy(gs[:, :], psgs[:, :])
                    grec = p2.tile([1, NCH], F32, tag="grec", name="grec")
                    nc.vector.reciprocal(grec[:, :], gs[:, :])
                    gmy = p2.tile([1, NCH], MM_DT, tag="gmy", name="gmy")
                    nc.vector.tensor_mul(gmy[:, :], eg[0:1, :], grec[:, :])
                    psG = psb(NCH)
                    nc.tensor.matmul(psG[:, :], lhsT=ones1_m[:, :],
                                     rhs=gmy[:, :], start=True, stop=True)
                    G = p2.tile([128, NCH], F32, tag="G", name="G")
                    nc.vector.tensor_copy(G[:, :], psG[:, :])
                    h_sb = fm_matmul(p2, w1_sb, 0, xall, FT, bias_tiles=b1,
                                     act=Act.Relu, tagpfx="mh")
                    for m in range(KT):
                        ps = psb(NCH)
                        for k in range(FT):
                            nc.tensor.matmul(
                                ps[:, :],
                                lhsT=w2_sb[k][:, m * 128:(m + 1) * 128],
                                rhs=h_sb[k][:, :], start=(k == 0),
                                stop=(k == FT - 1))
                        t = p2.tile([128, NCH], MM_DT, tag=f"ye{m}", name=f"ye{m}")
                        nc.scalar.activation(t[:, :], ps[:, :], Act.Identity,
                                             bias=b2[m][:, :])
                        nc.vector.tensor_mul(t[:, :], t[:, :], G[:, :])
                        nc.sync.dma_start(
                            rsin[r0 * D + m * 128:r0 * D + (m + 1) * 128, :],
                            t[:, 0:TOK])
                        nc.sync.dma_start(
                            rsin[r1 * D + m * 128:r1 * D + (m + 1) * 128, :],
                            t[:, TOK:NCH])
                nc.gpsimd.collective_compute(
                    "ReduceScatter", mybir.AluOpType.add,
                    replica_groups=ALL_GROUPS,
                    ins=[rsin.ap().opt()], outs=[rsout.ap().opt()])
                heat(30)
                z = []
                for k in range(KT):
                    tm = p.tile([128, TOK], MM_DT, tag=f"zl{k}", name=f"zl{k}")
                    nc.sync.dma_start(tm[:, :], rsout[k * 128:(k + 1) * 128, :])
                    t = p.tile([128, TOK], F32, tag=f"z{k}", name=f"z{k}")
                    nc.vector.tensor_add(t[:, :], tm[:, :], x_f32[k][:, :])
                    z.append(t)
                return layernorm(p, z, ln_idx, ofam)

        # ================= program =================
        heat(16)
        x = []
        for k in range(KT):
            t = actp.tile([128, TOK], F32, tag=f"x{k}", name=f"x{k}")
            nc.sync.dma_start(t[:, :], x0[k * 128:(k + 1) * 128, :])
            x.append(t)
        y = []
        for k in range(KT):
            t = actp.tile([128, TOK], F32, tag=f"y{k}", name=f"y{k}")
            nc.sync.dma_start(t[:, :], y0[k * 128:(k + 1) * 128, :])
            y.append(t)
        ekb_sb = []
        for k in range(S // 128):
            t = const.tile([128, 1], F32, tag=f"ekb{k}", name=f"ekb{k}")
            nc.sync.dma_start(t[:, :], ekb[k * 128:(k + 1) * 128, :])
            ekb_sb.append(t)
        dmask_sb = []
        for k in range(T // 128):
            t = const.tile([128, TOK], F32, tag=f"dm{k}", name=f"dm{k}")
            nc.sync.dma_start(t[:, :], dmask[k * 128:(k + 1) * 128, :])
            dmask_sb.append(t)

        # encoder
        for i in range(L):
            x_mm = cast_mm(x, "a")
            pair = pair_allgather(x_mm, f"e{i}")
            x = mha(x, x_mm, pair, "e", i, ekb_sb, None, 3 * i + 0, "x")
            x_mm = cast_mm(x, "b")
            x = ffn(x, x_mm, "e", i, 3 * i + 1, "x")
            x_mm = cast_mm(x, "a")
            x = moe(x, x_mm, i, 3 * i + 2, "x")

        # decoder
        enc_mm = cast_mm(x, "b")
        enc_pair = pair_allgather(enc_mm, "en")
        for i in range(L):
            y_mm = cast_mm(y, "a")
            ypair = pair_allgather(y_mm, f"d{i}")
            y = mha(y, y_mm, ypair, "d1", i, None, dmask_sb, 6 + 3 * i, "y")
            y_mm = cast_mm(y, "b")
            y = mha(y, y_mm, enc_pair, "d2", i, ekb_sb, None, 6 + 3 * i + 1,
                    "y")
            y_mm = cast_mm(y, "a")
            y = ffn(y, y_mm, "d", i, 6 + 3 * i + 2, "y")

        # final projection (vocab shard)
        y_mm = cast_mm(y, "b")
        fagin = nc.dram_tensor("fag_in", [D, TOK], MM_DT)
        fagout = nc.dram_tensor("fag_out", [NCORES * D, TOK], MM_DT,
                                addr_space="Shared")
        for k in range(KT):
            nc.sync.dma_start(fagin[k * 128:(k + 1) * 128, :], y_mm[k][:, :])
        nc.gpsimd.collective_compute(
            "AllGather", mybir.AluOpType.bypass, replica_groups=ALL_GROUPS,
            ins=[fagin.ap().opt()], outs=[fagout.ap().opt()])
        heat(14)
        with tc.tile_pool(name="finp", bufs=1) as p, \
             tc.tile_pool(name="finp2", bufs=2) as p2, \
             tc.tile_pool(name="finp3", bufs=4) as p3:
            fw_sb = []
            for k in range(KT):
                t = p.tile([128, VS], MM_DT, tag=f"fw{k}", name=f"fw{k}")
                nc.sync.dma_start(t[:, :], fw[k * 128:(k + 1) * 128, :])
                fw_sb.append(t)
            fbf = p.tile([1, VS], F32, tag="fbf", name="fbf")
            nc.sync.dma_start(fbf[:, :], fb[:, :])
            fb_sb = p.tile([1, VS], MM_DT, tag="fb", name="fb")
            nc.vector.tensor_copy(fb_sb[:, :], fbf[:, :])
            NV = 500
            for j in range(4):
                r0, r1 = 2 * j, 2 * j + 1
                yall = []
                for k in range(KT):
                    t = p2.tile([128, 2 * TOK], MM_DT, tag=f"ya{k}", name=f"ya{k}")
                    nc.sync.dma_start(
                        t[:, 0:TOK],
                        fagout[r0 * D + k * 128:r0 * D + (k + 1) * 128, :])
                    nc.sync.dma_start(
                        t[:, TOK:2 * TOK],
                        fagout[r1 * D + k * 128:r1 * D + (k + 1) * 128, :])
                    yall.append(t)
                for q in range(4):
                    orow = p3.tile([128, VS], F32, tag="of", name="of")
                    for n in range(VS // NV):
                        ps = psb(NV)
                        for k in range(KT):
                            nc.tensor.matmul(
                                ps[:, :],
                                lhsT=yall[k][:, q * 128:(q + 1) * 128],
                                rhs=fw_sb[k][:, n * NV:(n + 1) * NV],
                                start=(k == 0), stop=False)
                        nc.tensor.matmul(ps[:, :], lhsT=ones1_m[:, :],
                                         rhs=fb_sb[:, n * NV:(n + 1) * NV],
                                         start=False, stop=True)
                        nc.vector.tensor_copy(
                            orow[:, n * NV:(n + 1) * NV], ps[:, :])
                    nc.sync.dma_start(
                        out[j * 512 + q * 128:j * 512 + (q + 1) * 128, :],
                        orow[:, :])
        top.close()

    nc.compile()
    return nc


# ======================================================================
# host side
# ======================================================================

def _np(x):
    return np.asarray(x, dtype=np.float32)


def make_in_maps(inp, tar, params):
    inp = np.asarray(inp)
    tar = np.asarray(tar)
    pe = _pos_encoding(1000, D)
    xe = _np(params["emb_enc"])[inp] + pe[:S][None]      # [B,S,D]
    yd = _np(params["emb_dec"])[tar] + pe[:T][None]
    xe = xe.reshape(B * S, D)
    yd = yd.reshape(B * T, D)

    mmnp = (np.float32 if MM_DT in (F32, mybir.dt.float32r)
            else np.dtype("bfloat16"))

    def mm(a):
        return np.ascontiguousarray(_np(a)).astype(mmnp)

    def col(a):
        return np.ascontiguousarray(_np(a)[..., None])

    def pack_mha(p):
        w = np.concatenate([_np(p["wq"]) * 0.125, _np(p["wk"]), _np(p["wv"]),
                            _np(p["wo"])], axis=2)        # [L,D,4D]
        b = np.concatenate([_np(p["bq"]) * 0.125, _np(p["bk"]), _np(p["bv"]),
                            _np(p["bo"])], axis=1)        # [L,4D]
        return mm(w), col(b)

    wm_e, bm_e = pack_mha(params["enc_mha"])
    wm_d1, bm_d1 = pack_mha(params["dec_mha1"])
    wm_d2, bm_d2 = pack_mha(params["dec_mha2"])

    eln, dln, m = params["enc_ln"], params["dec_ln"], params["moe"]
    lng_ = np.stack([eln["g1"][0], eln["g2"][0], m["lng"][0],
                     eln["g1"][1], eln["g2"][1], m["lng"][1],
                     dln["g1"][0], dln["g2"][0], dln["g3"][0],
                     dln["g1"][1], dln["g2"][1], dln["g3"][1]])
    lnb_ = np.stack([eln["b1"][0], eln["b2"][0], m["lnb"][0],
                     eln["b1"][1], eln["b2"][1], m["lnb"][1],
                     dln["b1"][0], dln["b2"][0], dln["b3"][0],
                     dln["b1"][1], dln["b2"][1], dln["b3"][1]])

    look = 1.0 - np.tril(np.ones((T, T), np.float32))
    common = {
        "wmha_e": wm_e, "bmha_e": bm_e,
        "wmha_d1": wm_d1, "bmha_d1": bm_d1,
        "wmha_d2": wm_d2, "bmha_d2": bm_d2,
        "wf1_e": mm(params["enc_ffn"]["w1"]), "bf1_e": col(params["enc_ffn"]["b1"]),
        "wf2_e": mm(params["enc_ffn"]["w2"]), "bf2_e": col(params["enc_ffn"]["b2"]),
        "wf1_d": mm(params["dec_ffn"]["w1"]), "bf1_d": col(params["dec_ffn"]["b1"]),
        "wf2_d": mm(params["dec_ffn"]["w2"]), "bf2_d": col(params["dec_ffn"]["b2"]),
        "lng": col(lng_), "lnb": col(lnb_),
    }
    in_maps = []
    for c in range(NCORES):
        b, half = c // 2, c % 2
        rows = slice(c * TOK, (c + 1) * TOK)
        enc_pad = (inp[b] == 0).astype(np.float32)        # [S]
        dec_pad = (tar[b] == 0).astype(np.float32)        # [T]
        q0 = half * TOK
        comb = np.maximum(dec_pad[None, :], look[q0:q0 + TOK])  # [TOK,T]
        perm = (np.arange(E) + c) % E
        im = dict(common)
        im.update({
            "x0": np.ascontiguousarray(xe[rows].T),
            "y0": np.ascontiguousarray(yd[rows].T),
            "ekb": np.ascontiguousarray((enc_pad * NEGM)[:, None]),
            "dmask": np.ascontiguousarray((comb * NEGM).T),
            "gw": mm(_np(m["gw"])[:, :, perm]),
            "gb": col(_np(m["gb"])[:, perm]),
            "mw1": mm(_np(m["w1"])[:, c]),
            "mb1": col(_np(m["b1"])[:, c]),
            "mw2": mm(_np(m["w2"])[:, c]),
            "mb2": col(_np(m["b2"])[:, c]),
            "fw": mm(_np(params["final_w"])[:, c * VS:(c + 1) * VS]),
            "fb": np.ascontiguousarray(
                _np(params["final_b"])[None, c * VS:(c + 1) * VS]),
        })
        in_maps.append(im)
    return in_maps


_CACHED_NC = None


def get_nc():
    global _CACHED_NC
    if _CACHED_NC is None:
        _CACHED_NC = build_nc()
    return _CACHED_NC


def run(inp, tar, params, trace=False, tmpdir=None):
    nc = get_nc()
    in_maps = make_in_maps(inp, tar, params)
    res = bass_utils.run_bass_kernel_spmd(
        nc, in_maps, core_ids=list(range(NCORES)), trace=trace, tmpdir=tmpdir)
    full = np.concatenate([res.results[c]["out"] for c in range(NCORES)],
                          axis=1)                         # [2048, V]
    return np.ascontiguousarray(full.reshape(B, T, V)), res


def kernel(inp, tar, params):
    return run(inp, tar, params)[0]
